# revision 1
# baseline (speedup 1.0000x reference)
"""KAN layer (B=8192, IN_F=OUT_F=1024, GRID=5) on 8 Trainium2 cores.

Math: Y[b,o] = W0[o]*silu(x) + W1[o]*spline_o(clip(x,-1,1)) + b[o], x = X[b,o]
(idx_in = arange(O) % IN_F is the identity here since O == IN_F).

The degree-1 B-spline on the uniform 5-knot grid over [-1,1] is rewritten in
the relu basis: spline(xc) = c0 + m0*(xc+1) + sum_j (m_j - m_{j-1}) * relu(xc - s_j)
with slopes m_g = 2*(c_{g+1}-c_g) and interior knots s_j in {-0.5, 0, 0.5}.
Folding W1 and b gives  Y^T[o,:] = W0*silu + B'*xc + G1*r1 + G2*r2 + G3*r3 + A'.

Layout: edges on SBUF partitions (X pre-transposed on host), batch on the free
dim, data-parallel over batch across the 8 cores.  The per-edge weighted sum of
the 5 feature maps runs on TensorE as 5 diagonal-stationary fp32r matmuls
accumulating in PSUM; ScalarE evacuates PSUM adding the per-edge bias A'.
"""
import sys

for _p in ("/root/.axon_site", "/root/.axon_site/_ro/trn_rl_repo", "/root/.axon_site/_ro/pypackages"):
    if _p not in sys.path:
        sys.path.append(_p)

import numpy as np

import concourse.bacc as bacc
import concourse.tile as tile
from concourse import mybir
from concourse.bass_utils import run_bass_kernel_spmd

B, IN_F, OUT_F, GRID = 8192, 1024, 1024, 5
N_CORES = 8
B_SHARD = B // N_CORES          # 1024 batch rows per core
EB = OUT_F // 128               # 8 edge blocks
NF = 5                          # features: silu, xc, r1, r2, r3
CHUNK = 512                     # matmul moving free-dim (one PSUM bank, fp32)
NCHUNK = B_SHARD // CHUNK

_nc_cache = None


def _build():
    f32 = mybir.dt.float32
    f32r = mybir.dt.float32r
    nc = bacc.Bacc("TRN2", target_bir_lowering=False, debug=False)
    xt = nc.dram_tensor("xt", [OUT_F, B_SHARD], f32, kind="ExternalInput").ap()
    diag = nc.dram_tensor("diag", [128, EB * NF * 128], f32r, kind="ExternalInput").ap()
    apcol = nc.dram_tensor("apcol", [128, EB], f32, kind="ExternalInput").ap()
    yt = nc.dram_tensor("yt", [OUT_F, B_SHARD], f32, kind="ExternalOutput").ap()

    with tile.TileContext(nc) as tc:
        with tc.tile_pool(name="const", bufs=1) as const_pool, \
             tc.tile_pool(name="xin", bufs=3) as xin_pool, \
             tc.tile_pool(name="feat", bufs=2) as feat_pool, \
             tc.tile_pool(name="yout", bufs=3) as yout_pool, \
             tc.tile_pool(name="ps", bufs=4, space="PSUM") as psum_pool:
            diag_sb = const_pool.tile([128, EB * NF * 128], f32r)
            nc.sync.dma_start(diag_sb[:], diag[:, :])
            ap_sb = const_pool.tile([128, EB], f32)
            nc.sync.dma_start(ap_sb[:], apcol[:, :])

            for e in range(EB):
                x_t = xin_pool.tile([128, B_SHARD], f32, tag="x")
                nc.sync.dma_start(x_t[:], xt[e * 128:(e + 1) * 128, :])

                silu_t = feat_pool.tile([128, B_SHARD], f32r, tag="silu")
                nc.scalar.activation(silu_t[:], x_t[:], mybir.ActivationFunctionType.Silu)
                xc_t = feat_pool.tile([128, B_SHARD], f32r, tag="xc")
                nc.vector.tensor_scalar(xc_t[:], x_t[:], 1.0, -1.0,
                                        mybir.AluOpType.min, mybir.AluOpType.max)
                r1_t = feat_pool.tile([128, B_SHARD], f32r, tag="r1")
                nc.vector.tensor_scalar(r1_t[:], xc_t[:], 0.5, 0.0,
                                        mybir.AluOpType.add, mybir.AluOpType.max)
                r2_t = feat_pool.tile([128, B_SHARD], f32r, tag="r2")
                nc.vector.tensor_scalar_max(r2_t[:], xc_t[:], 0.0)
                r3_t = feat_pool.tile([128, B_SHARD], f32r, tag="r3")
                nc.vector.tensor_scalar(r3_t[:], xc_t[:], -0.5, 0.0,
                                        mybir.AluOpType.add, mybir.AluOpType.max)

                feats = (silu_t, xc_t, r1_t, r2_t, r3_t)
                for t in range(NCHUNK):
                    ps = psum_pool.tile([128, CHUNK], f32)
                    for f, ft in enumerate(feats):
                        lhsT = diag_sb[:, (e * NF + f) * 128:(e * NF + f + 1) * 128]
                        nc.tensor.matmul(ps[:], lhsT, ft[:, t * CHUNK:(t + 1) * CHUNK],
                                         start=(f == 0), stop=(f == NF - 1))
                    yo = yout_pool.tile([128, CHUNK], f32, tag="yo")
                    nc.scalar.activation(yo[:], ps[:], mybir.ActivationFunctionType.Identity,
                                         bias=ap_sb[:, e:e + 1], scale=1.0)
                    nc.sync.dma_start(yt[e * 128:(e + 1) * 128, t * CHUNK:(t + 1) * CHUNK], yo[:])
    nc.compile()
    return nc


def _host_prep(X, coeffs, W, b):
    c = coeffs.astype(np.float64)
    W = W.astype(np.float64)
    b = b.astype(np.float64)
    m = 2.0 * (c[:, 1:] - c[:, :-1])            # [O, 4] slopes per unit xc
    w1 = W[:, 1]
    aprime = w1 * (c[:, 0] + m[:, 0]) + b        # const term (incl. m0*(xc+1) fold)
    bprime = w1 * m[:, 0]
    g = w1[:, None] * (m[:, 1:] - m[:, :-1])     # [O, 3] relu weights at s=-0.5,0,0.5
    wvec = np.stack([W[:, 0], bprime, g[:, 0], g[:, 1], g[:, 2]], axis=1)  # [O, 5]

    diag = np.zeros((128, EB * NF * 128), dtype=np.float32)
    k = np.arange(128)
    for e in range(EB):
        for f in range(NF):
            diag[k, (e * NF + f) * 128 + k] = wvec[e * 128 + k, f].astype(np.float32)
    apcol = np.empty((128, EB), dtype=np.float32)
    for e in range(EB):
        apcol[:, e] = aprime[e * 128:(e + 1) * 128].astype(np.float32)
    return diag, apcol


def kernel(X, coeffs, W, b):
    global _nc_cache
    if _nc_cache is None:
        _nc_cache = _build()
    nc = _nc_cache

    diag, apcol = _host_prep(X, coeffs, W, b)
    in_maps = []
    for c in range(N_CORES):
        xt_shard = np.ascontiguousarray(X[c * B_SHARD:(c + 1) * B_SHARD, :].T)
        in_maps.append({"xt": xt_shard, "diag": diag, "apcol": apcol})

    res = run_bass_kernel_spmd(nc, in_maps, core_ids=list(range(N_CORES)))
    Y = np.empty((B, OUT_F), dtype=np.float32)
    for c in range(N_CORES):
        Y[c * B_SHARD:(c + 1) * B_SHARD, :] = res.results[c]["yt"].T
    return Y


# revision 2
# speedup vs baseline: 1.0645x; 1.0645x over previous
"""KAN layer (B=8192, IN_F=OUT_F=1024, GRID=5) on 8 Trainium2 cores.

Math: Y[b,o] = W0[o]*silu(x) + W1[o]*spline_o(clip(x,-1,1)) + b[o], x = X[b,o]
(idx_in = arange(O) % IN_F is the identity here since O == IN_F).

The degree-1 B-spline on the uniform 5-knot grid over [-1,1] is rewritten in
the relu basis: spline(xc) = c0 + m0*(xc+1) + sum_j (m_j - m_{j-1})*relu(xc - s_j)
with slopes m_g = 2*(c_{g+1}-c_g) and interior knots s_j in {-0.5, 0, 0.5}.
Folding W1 and b gives  Y^T[o,:] = W0*silu + B'*xc + G1*r1 + G2*r2 + G3*r3 + A'.

Layout: edges on SBUF partitions (X pre-transposed on host), batch on the free
dim, data-parallel over batch across the 8 cores.  The per-edge weighted sum of
the 5 feature maps runs on TensorE as 5 diagonal-stationary matmuls (silu in
fp32r, the four spline features in fp16) accumulating in PSUM; ScalarE
evacuates PSUM adding the per-edge bias A'.  Diagonal stationaries are built
on-device (identity * per-partition weight) to keep DMA traffic at the 8 MB
payload floor.
"""
import sys

for _p in ("/root/.axon_site", "/root/.axon_site/_ro/trn_rl_repo", "/root/.axon_site/_ro/pypackages"):
    if _p not in sys.path:
        sys.path.append(_p)

import numpy as np

import concourse.bacc as bacc
import concourse.tile as tile
from concourse import mybir
from concourse.bass_utils import run_bass_kernel_spmd

B, IN_F, OUT_F, GRID = 8192, 1024, 1024, 5
N_CORES = 8
B_SHARD = B // N_CORES          # 1024 batch rows per core
EB = OUT_F // 128               # 8 edge blocks
NF = 5                          # features: silu, xc, r1, r2, r3
CHUNK = 512                     # one PSUM bank of fp32

_nc_cache = None


def _build():
    f32 = mybir.dt.float32
    f32r = mybir.dt.float32r
    f16 = mybir.dt.float16
    AF = mybir.ActivationFunctionType
    OP = mybir.AluOpType
    nc = bacc.Bacc("TRN2", target_bir_lowering=False, debug=False)
    xt = nc.dram_tensor("xt", [OUT_F, B_SHARD], f32, kind="ExternalInput").ap()
    # const pack: [:, 0:128] identity fp32, [:, 128:168] wT (5 feature weights
    # per edge block), [:, 168:176] A' per edge block
    cpack = nc.dram_tensor("cpack", [128, 176], f32, kind="ExternalInput").ap()
    yt = nc.dram_tensor("yt", [OUT_F, B_SHARD], f32, kind="ExternalOutput").ap()

    xt3 = xt.rearrange("(n p) d -> p n d", p=128)   # [128, EB, B_SHARD]
    yt3 = yt.rearrange("(n p) d -> p n d", p=128)

    with tile.TileContext(nc) as tc:
        with tc.tile_pool(name="const", bufs=1) as const_pool, \
             tc.tile_pool(name="xin", bufs=3) as xin_pool, \
             tc.tile_pool(name="feat", bufs=2) as feat_pool, \
             tc.tile_pool(name="yout", bufs=2) as yout_pool, \
             tc.tile_pool(name="ps", bufs=3, space="PSUM") as psum_pool:
            cp = const_pool.tile([128, 176], f32)
            nc.sync.dma_start(cp[:], cpack[:, :])
            ident32 = cp[:, 0:128]
            ident16 = const_pool.tile([128, 128], f16)
            nc.vector.tensor_copy(ident16[:], ident32)

            # per-block diagonal stationaries, built on device
            dsilu = const_pool.tile([128, EB * 128], f32r)
            dspl = const_pool.tile([128, EB * 4 * 128], f16)
            for e in range(EB):
                nc.vector.tensor_scalar_mul(
                    dsilu[:, e * 128:(e + 1) * 128], ident32, cp[:, 128 + e * NF:128 + e * NF + 1])
                for j in range(4):
                    nc.vector.tensor_scalar_mul(
                        dspl[:, (e * 4 + j) * 128:(e * 4 + j + 1) * 128],
                        ident16[:], cp[:, 128 + e * NF + 1 + j:128 + e * NF + 2 + j])

            for ep in range(EB // 2):          # pairs of edge blocks
                x_t = xin_pool.tile([128, 2, B_SHARD], f32, tag="x")
                nc.sync.dma_start(x_t[:], xt3[:, 2 * ep:2 * ep + 2, :])
                for h in range(2):
                    e = 2 * ep + h
                    xe = x_t[:, h, :]
                    silu_t = feat_pool.tile([128, B_SHARD], f32r, tag="silu")
                    nc.scalar.activation(silu_t[:], xe, AF.Silu)
                    xc_t = feat_pool.tile([128, B_SHARD], f16, tag="xc")
                    nc.vector.tensor_scalar(xc_t[:], xe, 1.0, -1.0, OP.min, OP.max)
                    r1_t = feat_pool.tile([128, B_SHARD], f16, tag="r1")
                    nc.vector.tensor_scalar(r1_t[:], xc_t[:], 0.5, 0.0, OP.add, OP.max)
                    r2_t = feat_pool.tile([128, B_SHARD], f16, tag="r2")
                    nc.vector.tensor_scalar_max(r2_t[:], xc_t[:], 0.0)
                    r3_t = feat_pool.tile([128, B_SHARD], f16, tag="r3")
                    nc.vector.tensor_scalar(r3_t[:], xc_t[:], -0.5, 0.0, OP.add, OP.max)

                    ps = psum_pool.tile([128, B_SHARD], f32, tag="ps")
                    for t in range(2):
                        nc.tensor.matmul(ps[:, t * CHUNK:(t + 1) * CHUNK],
                                         dsilu[:, e * 128:(e + 1) * 128],
                                         silu_t[:, t * CHUNK:(t + 1) * CHUNK],
                                         start=True, stop=False, skip_group_check=True)
                    for j, ft in enumerate((xc_t, r1_t, r2_t, r3_t)):
                        for t in range(2):
                            nc.tensor.matmul(ps[:, t * CHUNK:(t + 1) * CHUNK],
                                             dspl[:, (e * 4 + j) * 128:(e * 4 + j + 1) * 128],
                                             ft[:, t * CHUNK:(t + 1) * CHUNK],
                                             start=False, stop=(j == 3), skip_group_check=True)

                    if h == 0:
                        yo = yout_pool.tile([128, 2, B_SHARD], f32, tag="yo")
                    nc.scalar.activation(yo[:, h, :], ps[:], AF.Identity,
                                         bias=cp[:, 168 + e:169 + e], scale=1.0)
                nc.scalar.dma_start(yt3[:, 2 * ep:2 * ep + 2, :], yo[:])
    nc.compile()
    return nc


def _host_prep(X, coeffs, W, b):
    c = coeffs.astype(np.float64)
    W = W.astype(np.float64)
    b = b.astype(np.float64)
    m = 2.0 * (c[:, 1:] - c[:, :-1])            # [O, 4] slopes per unit xc
    w1 = W[:, 1]
    aprime = w1 * (c[:, 0] + m[:, 0]) + b        # const term (incl. m0*(xc+1) fold)
    bprime = w1 * m[:, 0]
    g = w1[:, None] * (m[:, 1:] - m[:, :-1])     # [O, 3] relu weights at s=-0.5,0,0.5
    wvec = np.stack([W[:, 0], bprime, g[:, 0], g[:, 1], g[:, 2]], axis=1)  # [O, 5]

    cpack = np.zeros((128, 176), dtype=np.float32)
    cpack[:, 0:128] = np.eye(128, dtype=np.float32)
    for e in range(EB):
        for f in range(NF):
            cpack[:, 128 + e * NF + f] = wvec[e * 128:(e + 1) * 128, f].astype(np.float32)
        cpack[:, 168 + e] = aprime[e * 128:(e + 1) * 128].astype(np.float32)
    return cpack


def kernel(X, coeffs, W, b):
    global _nc_cache
    if _nc_cache is None:
        _nc_cache = _build()
    nc = _nc_cache

    cpack = _host_prep(X, coeffs, W, b)
    in_maps = []
    for c in range(N_CORES):
        xt_shard = np.ascontiguousarray(X[c * B_SHARD:(c + 1) * B_SHARD, :].T)
        in_maps.append({"xt": xt_shard, "cpack": cpack})

    res = run_bass_kernel_spmd(nc, in_maps, core_ids=list(range(N_CORES)))
    Y = np.empty((B, OUT_F), dtype=np.float32)
    for c in range(N_CORES):
        Y[c * B_SHARD:(c + 1) * B_SHARD, :] = res.results[c]["yt"].T
    return Y
